# revision 7
# baseline (speedup 1.0000x reference)
"""LNS Linear (y = x @ W^T + b) on 8 Trainium2 NeuronCores.

The reference computes the linear layer through log-number-system
arithmetic, which is mathematically identical to a plain fp32 matmul
plus bias (the LNS round-trip only changes float rounding, ~1e-5
scale-relative).  So the kernel is a distributed fp32 matmul:

  - 2D shard: 4 batch-blocks (128 rows) x 2 N-blocks (256 cols);
    core i handles (bi = i // 2, nj = i % 2).
  - Per core: out[128, 256] = x_blk[128, 512] @ w_blk[256, 512]^T + bias_blk
  - Both operands are stored K-contiguous, so each is transposed on-chip
    via PE-transpose (identity matmul) before the accumulating matmuls.
  - Bias is added with a rank-1 matmul (ones[1,128]^T @ bias[1,256])
    accumulating into the same PSUM bank.
"""

import numpy as np

import concourse.bass as bass
import concourse.bacc as bacc
import concourse.mybir as mybir
from concourse.bass_utils import run_bass_kernel_spmd
from concourse.masks import make_identity
from concourse.tile import TileContext

B, K, N = 512, 512, 512
N_CORES = 8
GRID_B, GRID_N = 4, 2
BB = B // GRID_B  # 128 batch rows per core
NB = N // GRID_N  # 256 output cols per core
P = 128
KT = K // P  # 4 k-tiles
NT = NB // P  # 2 n-tiles inside the per-core N block

_CACHE: dict = {}


def _build_program() -> bass.Bass:
    # Bacc (not plain Bass): its compile() pipeline runs
    # generate_event_semaphores, which splits multi-sem waits into the
    # single-wait form walrus codegen requires.
    nc = bacc.Bacc()
    x = nc.declare_dram_parameter("x", [BB, K], mybir.dt.float32, isOutput=False)
    w = nc.declare_dram_parameter("weight", [NB, K], mybir.dt.float32, isOutput=False)
    b = nc.declare_dram_parameter("bias", [NB], mybir.dt.float32, isOutput=False)
    out = nc.declare_dram_parameter("out", [BB, NB], mybir.dt.float32, isOutput=True)

    f32 = mybir.dt.float32
    with TileContext(nc) as tc:
        with (
            tc.tile_pool(name="const", bufs=1) as const,
            tc.tile_pool(name="nat", bufs=6) as nat,
            tc.tile_pool(name="trans", bufs=3) as trans,
            tc.tile_pool(name="tpsum", bufs=4, space="PSUM") as tpsum,
            tc.tile_pool(name="wpsum", bufs=1, space="PSUM") as wpsum,
            tc.tile_pool(name="opsum", bufs=1, space="PSUM") as opsum,
            tc.tile_pool(name="outp", bufs=1) as outp,
        ):
            identity = const.tile([P, P], f32)
            make_identity(nc, identity)
            ones = const.tile([1, P], f32)
            nc.gpsimd.memset(ones[:], 1.0)
            bias_sb = const.tile([1, NB], f32)
            nc.sync.dma_start(out=bias_sb[:], in_=b[None, :])

            # Throwaway PE transpose: makes the PE observe the gpsimd
            # constant-setup semaphore before any data arrives, so every
            # later PE instruction needs at most ONE sync wait (its DMA).
            # Walrus codegen only allows a single wait on the PE's
            # LDWEIGHTS slot.
            warm = wpsum.tile([P, P], f32)
            nc.tensor.transpose(warm[:], identity[:], identity[:])

            psum_out = opsum.tile([P, NB], f32)
            for kt in range(KT):
                # x^T k-tile: load [128,128] block, PE-transpose, copy to SBUF
                x_nat = nat.tile([P, P], f32, tag="xnat")
                nc.sync.dma_start(out=x_nat[:], in_=x[:, kt * P : (kt + 1) * P])
                pt = tpsum.tile([P, P], f32, tag="tp")
                nc.tensor.transpose(pt[:], x_nat[:], identity[:])
                xT = trans.tile([P, P], f32, tag="xT")
                nc.vector.tensor_copy(xT[:], pt[:])

                # w^T k-tile [128k, 256n]
                wT = trans.tile([P, NB], f32, tag="wT")
                for nt in range(NT):
                    w_nat = nat.tile([P, P], f32, tag="wnat")
                    nc.sync.dma_start(
                        out=w_nat[:],
                        in_=w[nt * P : (nt + 1) * P, kt * P : (kt + 1) * P],
                    )
                    pw = tpsum.tile([P, P], f32, tag="tp")
                    nc.tensor.transpose(pw[:], w_nat[:], identity[:])
                    nc.vector.tensor_copy(wT[:, nt * P : (nt + 1) * P], pw[:])

                nc.tensor.matmul(
                    psum_out[:], xT[:], wT[:], start=(kt == 0), stop=False
                )

            # bias via rank-1 matmul accumulating into the same PSUM bank
            nc.tensor.matmul(psum_out[:], ones[:], bias_sb[:], start=False, stop=True)

            out_sb = outp.tile([P, NB], f32)
            nc.vector.tensor_copy(out_sb[:], psum_out[:])
            nc.sync.dma_start(out=out[:, :], in_=out_sb[:])

    # Bacc.finalize runs the lowering pipeline (register allocation,
    # event-semaphore wait splitting) that walrus codegen requires.
    nc.finalize()
    return nc


def _get_program() -> bass.Bass:
    if "nc" not in _CACHE:
        _CACHE["nc"] = _build_program()
    return _CACHE["nc"]


def _shard(x: np.ndarray, weight: np.ndarray, bias: np.ndarray):
    in_maps = []
    for core in range(N_CORES):
        bi, nj = core // GRID_N, core % GRID_N
        in_maps.append(
            {
                "x": np.ascontiguousarray(x[bi * BB : (bi + 1) * BB, :]),
                "weight": np.ascontiguousarray(weight[nj * NB : (nj + 1) * NB, :]),
                "bias": np.ascontiguousarray(bias[nj * NB : (nj + 1) * NB]),
            }
        )
    return in_maps


def kernel(x, weight, bias) -> np.ndarray:
    x = np.asarray(x, dtype=np.float32)
    weight = np.asarray(weight, dtype=np.float32)
    bias = np.asarray(bias, dtype=np.float32)

    nc = _get_program()
    in_maps = _shard(x, weight, bias)
    results = run_bass_kernel_spmd(nc, in_maps, list(range(N_CORES))).results

    out = np.empty((B, N), dtype=np.float32)
    for core in range(N_CORES):
        bi, nj = core // GRID_N, core % GRID_N
        out[bi * BB : (bi + 1) * BB, nj * NB : (nj + 1) * NB] = results[core]["out"]
    return out
